# revision 1
# baseline (speedup 1.0000x reference)
"""SwitchBack global-quantized MLP on 8 TRN2 NeuronCores, v2.

Data-parallel over tokens (1024 rows/core).  mm1 computes X2_pre TRANSPOSED
(H on partitions, tokens on free axis) by using W1 tiles as the stationary
operand and quantized-transposed activations as the moving operand.  This
kills the X2 DRAM roundtrip and all X2 transposes: gelu output stays in SBUF
(bf16) and its tiles are directly mm2's stationary operand layout.

Per-token scales live on the free axis in H-major space; they are broadcast
across partitions with tiny K=1 matmuls (ones-vector outer product).
Row maxes (needed per token across partitions) are computed by elementwise
max/min accumulation over h-tiles + PE-transpose + free-axis reduce.

Weights are globally int8-quantized on the host (numerically identical to
the reference) and shipped pre-transposed AND pre-tiled in bf16 so every
weight DMA is a large fully-contiguous transfer.
"""

import numpy as np
import ml_dtypes

import concourse.bass as bass
import concourse.mybir as mybir
import concourse.tile as tile
from concourse import bacc
from concourse.bass_utils import run_bass_kernel_spmd

Q = 127.0
MAGIC = 12582912.0  # 1.5 * 2**23: (v + MAGIC) - MAGIC == RNE-round(v), |v| <= 2**22
P = 128
FD = 512

F32 = mybir.dt.float32
BF16 = mybir.dt.bfloat16


def build_program(NR, D, H, c1, c2, n_cores=8, reps=1, ring_extra=1,
                  w1_bufs=2, w2_kc=4, w2_bufs=3):
    """One-core SPMD program.  NR tokens, D model dim, H hidden dim.
    c1/c2 = sW/(Q*Q) global dequant consts."""
    MT, KD, NH, ND = NR // P, D // P, H // P, D // FD
    TF = NR // FD            # mm1 free-dim chunks of tokens
    MPF = FD // P            # m-tiles per free-dim chunk
    AF = mybir.ActivationFunctionType
    OP = mybir.AluOpType
    RING = NH + ring_extra

    nc = bacc.Bacc("TRN2", target_bir_lowering=False, debug=False,
                   num_devices=n_cores)
    x_d = nc.dram_tensor("x", [NR, D], F32, kind="ExternalInput")
    # w1t: [NH, 128, KD*128] bf16; per h-tile one contiguous [128, D] block,
    # partition p holds W1qT[k*128+p, h*128+j] at free offset k*128+j.
    w1_d = nc.dram_tensor("w1t", [NH, P, D], BF16, kind="ExternalInput")
    # w2t: [ND, 128, NH*512]; partition p holds W2qT[k*128+p, d*512+j] at
    # free offset k*512+j.
    w2_d = nc.dram_tensor("w2t", [ND, P, NH * FD], BF16, kind="ExternalInput")
    b1_d = nc.dram_tensor("b1t", [P, NH], F32, kind="ExternalInput")  # cols
    b2_d = nc.dram_tensor("b2r", [P, D], BF16, kind="ExternalInput")  # bcast
    id_d = nc.dram_tensor("idf", [P, P], F32, kind="ExternalInput")
    ones_d = nc.dram_tensor("onesr", [1, P], F32, kind="ExternalInput")
    out_d = nc.dram_tensor("out", [NR, D], F32, kind="ExternalOutput")

    with tile.TileContext(nc) as tc:
        with tc.tile_pool(name="psum", bufs=8, space="PSUM") as pp:
            for rep in range(reps):
                with tc.tile_pool(name="pg", bufs=1) as pg:
                    b1c = pg.tile([P, NH], F32, tag="b1c", name="b1c")
                    nc.sync.dma_start(out=b1c, in_=b1_d[:, :])
                    idf = pg.tile([P, P], F32, tag="idf", name="idf")
                    nc.sync.dma_start(out=idf, in_=id_d[:, :])
                    ones = pg.tile([1, P], F32, tag="ones", name="ones")
                    nc.sync.dma_start(out=ones, in_=ones_d[0:1, :])
                    am_col = pg.tile([P, MT], F32, tag="amc", name="am_col")
                    s2bc = pg.tile([P, NR], F32, tag="s2bc", name="s2bc")
                    ds2col = pg.tile([P, MT], F32, tag="ds2c", name="ds2col")
                    x1T = pg.tile([P, MT, KD, P], BF16, tag="x1T",
                                  name="x1T")
                    W1PRE = 3
                    w1pre = []
                    for h in range(W1PRE):
                        wp = pg.tile([P, KD, P], BF16, tag=f"w1p{h}",
                                     name=f"w1pre{h}")
                        nc.sync.dma_start(out=wp, in_=w1_d[h, :, :])
                        w1pre.append(wp)

                    # ---------- phase A: pipelined quantize + transpose ---
                    with tc.tile_pool(name="pa", bufs=1) as pa:
                        xts, s1ms, x1qs = [], [], []
                        for m in range(MT):
                            xt = pa.tile([P, D], F32, tag="xt",
                                         name=f"xt{m}", bufs=MT)
                            nc.sync.dma_start(
                                out=xt, in_=x_d[m * P:(m + 1) * P, :])
                            nc.vector.tensor_reduce(
                                am_col[:, m:m + 1], xt,
                                axis=mybir.AxisListType.X, op=OP.max,
                                apply_absolute_value=True)
                            rr = pa.tile([P, 1], F32, tag="rr",
                                         name=f"rr{m}")
                            nc.vector.reciprocal(rr, am_col[:, m:m + 1])
                            s1m = pa.tile([P, 1], F32, tag="s1m",
                                          name=f"s1m{m}")
                            nc.vector.tensor_scalar_mul(s1m, rr, Q)
                            xts.append(xt)
                            s1ms.append(s1m)
                        for m in range(MT):
                            nc.scalar.activation(xts[m], xts[m], AF.Copy,
                                                 bias=MAGIC, scale=s1ms[m])
                            x1q = pa.tile([P, D], BF16, tag="x1q",
                                          name=f"x1q{m}", bufs=MT)
                            nc.vector.tensor_scalar_sub(x1q, xts[m], MAGIC)
                            x1qs.append(x1q)
                            nc.sync.dma_start_transpose(
                                out=x1T[:, m, :, :], in_=x1q)

                    with tc.tile_pool(name="ring", bufs=RING) as rg:
                        g = []
                        with tc.tile_pool(name="pb", bufs=1) as pb:
                            # ds1bc = broadcast(am * c1) over partitions
                            ds1bc = pb.tile([P, NR], F32, tag="ds1bc",
                                            name="ds1bc")
                            ds1col = pb.tile([P, MT], F32, tag="ds1c",
                                             name="ds1col")
                            nc.vector.tensor_scalar_mul(ds1col, am_col, c1)
                            pt = pp.tile([MT, P], F32, tag="ps", name="ds1T")
                            nc.tensor.transpose(pt, ds1col, idf)
                            ds1r = pb.tile([MT, P], F32, tag="ds1r",
                                           name="ds1r")
                            nc.vector.tensor_copy(ds1r, pt)
                            ds1row = pb.tile([1, NR], F32, tag="rows",
                                             name="ds1row", bufs=3)
                            for m in range(MT):
                                nc.scalar.dma_start(
                                    out=ds1row[0:1, m * P:(m + 1) * P],
                                    in_=ds1r[m:m + 1, :])
                            for b in range(TF):
                                pbb = pp.tile([P, FD], F32, tag="ps",
                                              name=f"ds1b{b}")
                                nc.tensor.matmul(
                                    pbb, ones,
                                    ds1row[0:1, b * FD:(b + 1) * FD],
                                    start=True, stop=True)
                                nc.vector.tensor_copy(
                                    ds1bc[:, b * FD:(b + 1) * FD], pbb)

                            mxa = pb.tile([P, NR], BF16, tag="mxa",
                                          name="mxa")
                            mna = pb.tile([P, NR], BF16, tag="mna",
                                          name="mna")
                            nc.vector.memset(mxa, 0.0)
                            nc.vector.memset(mna, 0.0)

                            # ------ mm1 + gelu + max/min accumulate -------
                            for h in range(NH):
                                if h < W1PRE:
                                    w1 = w1pre[h]
                                else:
                                    w1 = pb.tile([P, KD, P], BF16, tag="w1",
                                                 name=f"w1_{h}",
                                                 bufs=w1_bufs)
                                    nc.sync.dma_start(out=w1,
                                                      in_=w1_d[h, :, :])
                                pss = [pp.tile([P, FD], F32, tag="ps",
                                               name=f"psA{h}_{t}")
                                       for t in range(TF)]
                                for k in range(KD):
                                    for t in range(TF):
                                        nc.tensor.matmul(
                                            pss[t], w1[:, k, :],
                                            x1T[:, t * MPF:(t + 1) * MPF,
                                                k, :],
                                            start=(k == 0),
                                            stop=(k == KD - 1))
                                gh = rg.tile([P, NR], BF16, tag="gq",
                                             name=f"g{h}")
                                for t in range(TF):
                                    nc.vector.tensor_tensor(
                                        pss[t], pss[t],
                                        ds1bc[:, t * FD:(t + 1) * FD],
                                        OP.mult)
                                    nc.scalar.activation(
                                        gh[:, t * FD:(t + 1) * FD], pss[t],
                                        AF.Gelu_apprx_tanh,
                                        bias=b1c[:, h:h + 1])
                                nc.vector.tensor_tensor(mxa, mxa, gh,
                                                        OP.max)
                                nc.vector.tensor_tensor(mna, mna, gh,
                                                        OP.min)
                                g.append(gh)

                            # ------ row maxes -> requant scales -----------
                            nc.vector.tensor_scalar_mul(mna, mna, -1.0)
                            nc.vector.tensor_tensor(mxa, mxa, mna, OP.max)
                            rmaxr = pb.tile([1, NR], F32, tag="rows",
                                            name="rmaxr", bufs=3)
                            nc.gpsimd.tensor_reduce(
                                rmaxr, mxa, axis=mybir.AxisListType.C,
                                op=OP.max)
                            s2row = pb.tile([1, NR], F32, tag="rows",
                                            name="s2row", bufs=3)
                            nc.vector.reciprocal(s2row, rmaxr)
                            nc.vector.tensor_scalar_mul(s2row, s2row, Q)
                            for b in range(TF):
                                pbb = pp.tile([P, FD], F32, tag="ps",
                                              name=f"s2b{b}")
                                nc.tensor.matmul(
                                    pbb, ones,
                                    s2row[0:1, b * FD:(b + 1) * FD],
                                    start=True, stop=True)
                                nc.vector.tensor_copy(
                                    s2bc[:, b * FD:(b + 1) * FD], pbb)
                            # ds2col (per-token dequant, tokens on partitions)
                            ds2row = pb.tile([1, NR], F32, tag="rows",
                                             name="ds2row", bufs=3)
                            nc.vector.tensor_scalar_mul(ds2row, rmaxr, c2)
                            pdc = pp.tile([P, MT], F32, tag="ps",
                                          name="ds2T")
                            for m in range(MT):
                                nc.tensor.transpose(
                                    pdc[:, m:m + 1],
                                    ds2row[0:1, m * P:(m + 1) * P],
                                    ones[0:1, 0:1])
                            nc.vector.tensor_copy(ds2col, pdc)

                        # ------ requant + mm2 + dequant + store -----------
                        with tc.tile_pool(name="pc", bufs=1) as pc:
                            b2bc = pc.tile([P, D], BF16, tag="b2bc",
                                           name="b2bc")
                            nc.scalar.dma_start(out=b2bc, in_=b2_d[:, :])
                            q2 = []
                            for h in range(NH):
                                tq2 = pc.tile([P, NR], F32, tag="tq2",
                                              name=f"tq2_{h}", bufs=2)
                                nc.vector.tensor_tensor(tq2, g[h], s2bc,
                                                        OP.mult)
                                qh = rg.tile([P, NR], BF16, tag="gq",
                                             name=f"q2T{h}")
                                nc.vector.tensor_scalar(qh, tq2, MAGIC,
                                                        MAGIC, op0=OP.add,
                                                        op1=OP.subtract)
                                q2.append(qh)

                            for d in range(ND):
                                pso = [pp.tile([P, FD], F32, tag="ps",
                                               name=f"psC{d}_{m}")
                                       for m in range(MT)]
                                for kc in range(NH // w2_kc):
                                    w2 = pc.tile([P, w2_kc, FD], BF16,
                                                 tag="w2",
                                                 name=f"w2_{d}_{kc}",
                                                 bufs=w2_bufs)
                                    nc.scalar.dma_start(
                                        out=w2,
                                        in_=w2_d[d][:, kc * w2_kc * FD:
                                                    (kc + 1) * w2_kc * FD]
                                        .rearrange("p (a j) -> p a j",
                                                   a=w2_kc))
                                    for kk in range(w2_kc):
                                        k = kc * w2_kc + kk
                                        for m in range(MT):
                                            nc.tensor.matmul(
                                                pso[m],
                                                q2[k][:, m * P:(m + 1) * P],
                                                w2[:, kk, :],
                                                start=(k == 0),
                                                stop=(k == NH - 1))
                                for m in range(MT):
                                    o = pc.tile([P, FD], F32, tag="o",
                                                name=f"o{d}_{m}", bufs=2)
                                    nc.vector.scalar_tensor_tensor(
                                        o, pso[m], ds2col[:, m:m + 1],
                                        b2bc[:, d * FD:(d + 1) * FD],
                                        op0=OP.mult, op1=OP.add)
                                    nc.scalar.dma_start(
                                        out=out_d[m * P:(m + 1) * P,
                                                  d * FD:(d + 1) * FD],
                                        in_=o)
    nc.compile()
    return nc


def _host_prep(x, W1, B1, W2, B2, n_cores=8):
    B, S, D = x.shape
    H = W1.shape[0]
    N = B * S
    NR = N // n_cores
    MT, KD, NH, ND = NR // P, D // P, H // P, D // FD
    X = np.ascontiguousarray(x.reshape(N, D))

    def quant_global_T(w):
        am = np.float32(np.max(np.abs(w)))
        scale = np.float32(Q) / am
        q = np.round(w.astype(np.float32) * scale)
        return np.ascontiguousarray(q.T).astype(ml_dtypes.bfloat16), am

    W1qT, sW1 = quant_global_T(W1)  # [D, H]
    W2qT, sW2 = quant_global_T(W2)  # [H, D]
    c1 = float(sW1) / (Q * Q)
    c2 = float(sW2) / (Q * Q)
    # w1t[h, p, k*128+j] = W1qT[k*128+p, h*128+j]
    w1t = np.ascontiguousarray(
        W1qT.reshape(KD, P, NH, P).transpose(2, 1, 0, 3).reshape(NH, P, D))
    # w2t[d, p, k*512+j] = W2qT[k*128+p, d*512+j]
    w2t = np.ascontiguousarray(
        W2qT.reshape(NH, P, ND, FD).transpose(2, 1, 0, 3).reshape(
            ND, P, NH * FD))
    b1t = np.ascontiguousarray(
        B1.astype(np.float32).reshape(NH, P).T)  # [128, NH]
    b2r = np.ascontiguousarray(np.broadcast_to(
        B2.astype(np.float32), (P, D))).astype(ml_dtypes.bfloat16)
    idf = np.eye(P, dtype=np.float32)
    onesr = np.ones((1, P), dtype=np.float32)

    in_maps = [
        {"x": X[i * NR:(i + 1) * NR], "w1t": w1t, "w2t": w2t,
         "b1t": b1t, "b2r": b2r, "idf": idf, "onesr": onesr}
        for i in range(n_cores)
    ]
    return in_maps, NR, D, H, c1, c2


def _run_sharded(nc, in_maps, n_cores, bench_iters=0):
    """Mirror bass2jax.run_bass_via_pjrt's multi-core path, with an optional
    steady-state timing loop over device-resident inputs."""
    import time

    import jax
    from jax.sharding import Mesh, NamedSharding, PartitionSpec
    from jax.experimental.shard_map import shard_map
    import concourse.mybir as mybir_
    from concourse import bass2jax

    bass2jax.install_neuronx_cc_hook()

    partition_name = (nc.partition_id_tensor.name
                      if nc.partition_id_tensor else None)
    in_names, out_names, out_avals, zero_outs = [], [], [], []
    for alloc in nc.m.functions[0].allocations:
        if not isinstance(alloc, mybir_.MemoryLocationSet):
            continue
        name = alloc.memorylocations[0].name
        if alloc.kind == "ExternalInput":
            if name != partition_name:
                in_names.append(name)
        elif alloc.kind == "ExternalOutput":
            out_names.append(name)
            shape = tuple(alloc.tensor_shape)
            dtype = mybir_.dt.np(alloc.dtype)
            out_avals.append(jax.core.ShapedArray(shape, dtype))
            zero_outs.append(np.zeros(shape, dtype))
    n_params = len(in_names)
    n_outs = len(out_avals)
    in_names = in_names + out_names
    if partition_name is not None:
        in_names.append(partition_name)
    donate = tuple(range(n_params, n_params + n_outs))

    def _body(*args):
        operands = list(args)
        if partition_name is not None:
            operands.append(bass2jax.partition_id_tensor())
        return tuple(bass2jax._bass_exec_p.bind(
            *operands,
            out_avals=tuple(out_avals),
            in_names=tuple(in_names),
            out_names=tuple(out_names),
            lowering_input_output_aliases=(),
            sim_require_finite=True,
            sim_require_nnan=True,
            nc=nc,
        ))

    devices = jax.devices()[:n_cores]
    mesh = Mesh(np.asarray(devices), ("core",))
    spec = NamedSharding(mesh, PartitionSpec("core"))
    sharded = jax.jit(
        shard_map(_body, mesh=mesh,
                  in_specs=(PartitionSpec("core"),) * (n_params + n_outs),
                  out_specs=(PartitionSpec("core"),) * n_outs,
                  check_rep=False),
        donate_argnums=donate, keep_unused=True)

    concat_in = [
        np.concatenate([np.asarray(in_maps[c][name]) for c in range(n_cores)],
                       axis=0)
        for name in in_names[:n_params]
    ]
    dev_in = [jax.device_put(a, spec) for a in concat_in]
    big_zeros = [np.zeros((n_cores * z.shape[0], *z.shape[1:]), z.dtype)
                 for z in zero_outs]

    def fresh_zeros():
        return [jax.device_put(z, spec) for z in big_zeros]

    out_arrs = sharded(*dev_in, *fresh_zeros())
    jax.block_until_ready(out_arrs)

    per_iter_s = None
    if bench_iters > 1:
        zero_sets = [fresh_zeros() for _ in range(bench_iters)]
        jax.block_until_ready(zero_sets)
        t0 = time.perf_counter()
        last = None
        for k in range(bench_iters):
            last = sharded(*dev_in, *zero_sets[k])
        jax.block_until_ready(last)
        per_iter_s = (time.perf_counter() - t0) / bench_iters

    results = [
        {name: np.asarray(out_arrs[i]).reshape(n_cores, *out_avals[i].shape)[c]
         for i, name in enumerate(out_names)}
        for c in range(n_cores)
    ]
    return results, per_iter_s


def kernel_with_results(x, W1, B1, W2, B2, bench_iters=0, reps=1):
    n_cores = 8
    in_maps, NR, D, H, c1, c2 = _host_prep(x, W1, B1, W2, B2, n_cores)
    nc = build_program(NR, D, H, c1, c2, n_cores, reps=reps)
    results, per_iter_s = _run_sharded(nc, in_maps, n_cores, bench_iters)
    out = np.concatenate([r["out"] for r in results], axis=0)
    return out.reshape(x.shape).astype(np.float32), per_iter_s


def kernel(x, W1, B1, W2, B2):
    return kernel_with_results(x, W1, B1, W2, B2)[0]

